# revision 22
# baseline (speedup 1.0000x reference)
"""Multi-head attention Bass/Tile kernel for TRN2, 8-core SPMD.

Sharding: core c handles batch b = c//2 and head-group g = c%2 (6 of 12 heads).
Each core computes its 6 heads end-to-end plus a partial output projection
(over its 384 of 768 ctx dims); the host sums the two partials per batch.

Design: all matmuls bf16 (PSUM accumulation stays f32). The ScalarE exp
stream (192 x [128,1024] ~ 218us) and the PE stream are co-critical:
  - score pairs K=64 run row-tiled (two heads concurrent on row groups)
  - ctx pairs M=64 run col-tiled (two heads concurrent on col groups of
    ONE psum bank)
  - softmax denominators: the DVE sums exp tiles pairwise then quadwise
    (bf16 2x mode), and one 4-way col-tiled M=32 ones-matmul group per
    FOUR steps accumulates Z replicated x32 per (head, s-half) quadrant
  - ctx/den trail the exp stream by 3 steps so semaphores pre-satisfy and
    psum bank recycling (1 ctx bank + 1 den bank) never stalls the PE
  - per-pair norm: Z quadrants cast to SBUF (stock copies; custom-DVE ops
    silently no-op off partition-base 0 on HW), selector matmuls (1/32
    averaging weights) assemble the 128-partition broadcast in psum, one
    base-0 fast reciprocal inverts it, one [128,512] multiply scales the
    evacuated ctx into ctxT
  - lead-in: wk/hs0/wq/wv then biases DMA'd first, PE warmup in the idle
    ctx bank during the DMA window, exp table preloaded at t=0
  - out-proj fused per s-block into later combos' background slots; the
    final s-block runs f=0,1 partials early and only its f=2 matmul plus
    one DVE add per 128-row chunk after the last normalization
"""

from collections import deque
from contextlib import ExitStack

import numpy as np
import ml_dtypes

import concourse.bass as bass
import concourse.tile as tile
from concourse import bacc, mybir
from concourse._compat import with_exitstack
from concourse.dve_ops import RECIPROCAL_APPROX_FAST, RECIP_APPROX_FAST_CONSTS

F32 = mybir.dt.float32
BF16 = mybir.dt.bfloat16
AF = mybir.ActivationFunctionType

B, E, S, H, D = 4, 768, 2048, 12, 64
NH = 6          # heads per core
HD = NH * D     # 384 head-dims per core
NE = E // 128   # 6 e-chunks
NM = HD // 128  # 3 m-chunks (2 heads each)
NT = S // 128   # 16 t-tiles
SBW = 512       # s-block width
NS = S // SBW   # 4 s-blocks
HB = SBW // 2   # den split width (256)
WARMUP = 12
PEND = 3        # ctx/den trail the exp stream by this many steps


@with_exitstack
def mha_tile(ctx: ExitStack, tc, hs, wq, wk, wv, bq, bk, bv, woT, bo2, outT):
    nc = tc.nc

    persist = ctx.enter_context(tc.tile_pool(name="persist", bufs=1))

    # --- persistent SBUF tiles (packed layouts match the DRAM packing);
    # hs is s-chunked: the first score/exp work only needs chunk 0 ---
    hs_sb = [persist.tile([128, NE, SBW], BF16, name=f"hs{c}") for c in range(NS)]
    wq_sb = persist.tile([128, NE, HD], BF16, name="wq")
    wk_sb = persist.tile([128, NE, HD], BF16, name="wk")
    wv_sb = persist.tile([128, NE, HD], BF16, name="wv")
    woT_sb = persist.tile([128, NM, E], BF16, name="wo")
    qT_sb = [persist.tile([128, S], BF16, name=f"qT{m}") for m in range(NM)]
    kT_sb = [persist.tile([128, S], BF16, name=f"kT{m}") for m in range(NM)]
    ctxT_sb = [persist.tile([128, S], BF16, name=f"ctxT{m}") for m in range(NM)]
    v_sb = [persist.tile([128, NH, D], BF16, name=f"v{t}") for t in range(NT)]

    bq_sb = persist.tile([128, NM], F32, name="bq")
    bk_sb = persist.tile([128, NM], F32, name="bk")
    bv_bc = persist.tile([128, HD], F32, name="bv")
    bo_sb = persist.tile([128, NE], F32, name="bo")
    ones32 = persist.tile([128, 32], BF16, name="ones32")
    zs = persist.tile([128, SBW], BF16, name="zs")
    sels = [persist.tile([128, 64], BF16, name=f"sel{r}") for r in (0, 32, 64, 96)]
    scratch = persist.tile([128, 256], BF16, name="scratch")
    scon = persist.tile([128, 256], F32, name="scon")
    preload = persist.tile([1, 1], F32, name="preload")

    # --- DMA: first-score dependencies first (wk, hs chunk 0, wq), then the
    # rest in the order the background units need them ---
    nc.sync.dma_start(wk_sb[:], wk)
    nc.sync.dma_start(hs_sb[0][:], hs[:, 0, :, :])
    nc.sync.dma_start(wq_sb[:], wq)
    nc.sync.dma_start(wv_sb[:], wv)
    nc.sync.dma_start(bk_sb[:], bk.rearrange("(m p) -> p m", p=128))
    nc.sync.dma_start(bq_sb[:], bq.rearrange("(m p) -> p m", p=128))
    nc.sync.dma_start(
        bv_bc[:], bass.AP(tensor=bv.tensor, offset=bv.offset, ap=[[0, 128], [1, HD]])
    )
    nc.sync.dma_start(bo_sb[:], bo2.rearrange("(m p) -> p m", p=128))
    nc.sync.dma_start(hs_sb[1][:], hs[:, 1, :, :])
    nc.sync.dma_start(hs_sb[2][:], hs[:, 2, :, :])
    nc.sync.dma_start(hs_sb[3][:], hs[:, 3, :, :])
    nc.sync.dma_start(woT_sb[:], woT)

    # scratch first: warmup matmuls gate only on this one memset
    nc.vector.memset(scratch[:], 0.0)
    nc.vector.memset(ones32[:], 1.0)
    # big constant inits on the otherwise-idle gpsimd engine
    nc.gpsimd.memset(zs[:], 0.0)
    for sel, r in zip(sels, (0, 32, 64, 96)):
        nc.gpsimd.memset(sel[:], 0.0)
        nc.gpsimd.memset(sel[r : r + 32, :], 1.0 / 32.0)

    # exp table preload: a 1-element activation at t=0 hoists the ~2.7us
    # ACT_TABLE_LOAD off the first real exp
    nc.scalar.activation(preload[:], scratch[0:1, 0:1], AF.Exp)

    # --- PSUM pools (8 banks: 4 sc + 1 ctx + 1 den + 2 misc) ---
    pssc = ctx.enter_context(tc.tile_pool(name="pssc", bufs=2, space="PSUM"))
    psctx = ctx.enter_context(tc.tile_pool(name="psctx", bufs=1, space="PSUM"))
    psden = ctx.enter_context(tc.tile_pool(name="psden", bufs=1, space="PSUM"))
    psmisc = ctx.enter_context(tc.tile_pool(name="psmisc", bufs=2, space="PSUM"))

    # SBUF working pools
    expp = ctx.enter_context(tc.tile_pool(name="expp", bufs=6))
    esump = ctx.enter_context(tc.tile_pool(name="esump", bufs=3))
    pbp = ctx.enter_context(tc.tile_pool(name="pbp", bufs=6))
    cup = ctx.enter_context(tc.tile_pool(name="cup", bufs=2))
    bcip = ctx.enter_context(tc.tile_pool(name="bcip", bufs=2))
    outp = ctx.enter_context(tc.tile_pool(name="outp", bufs=4))

    # --- PE warmup during the DMA window: ramps the tensor engine HAM to
    # full p-state before real work. Outputs consumed to keep walrus honest.
    # warmup runs in the psctx bank (idle until combo 0 step 3) so the
    # first k/q units never queue behind warmup's psum WAR chain
    for i in range(WARMUP):
        wp = psctx.tile([128, 2 * 256], F32, tag="ctx", name="cp")
        nc.tensor.matmul(wp[:, 0:256], scratch[:, 0:128], scratch[:],
                         start=True, stop=True)
        if i % 8 == 7:
            nc.vector.tensor_copy(scon[:], wp[:, 0:256])

    # --- work units ---
    def q_unit(m, s):
        msl = slice(128 * m, 128 * (m + 1))
        ssl = slice(SBW * s, SBW * (s + 1))
        qp = psmisc.tile([128, SBW], F32, tag="misc")
        for e in range(NE):
            nc.tensor.matmul(
                qp[:], wq_sb[:, e, msl], hs_sb[s][:, e, :],
                start=(e == 0), stop=(e == NE - 1),
            )
        nc.vector.tensor_scalar_add(
            out=qT_sb[m][:, ssl], in0=qp[:], scalar1=bq_sb[:, m : m + 1]
        )

    def k_unit(m, s):
        msl = slice(128 * m, 128 * (m + 1))
        ssl = slice(SBW * s, SBW * (s + 1))
        kp = psmisc.tile([128, SBW], F32, tag="misc")
        for e in range(NE):
            nc.tensor.matmul(
                kp[:], wk_sb[:, e, msl], hs_sb[s][:, e, :],
                start=(e == 0), stop=(e == NE - 1),
            )
        nc.vector.tensor_scalar_add(
            out=kT_sb[m][:, ssl], in0=kp[:], scalar1=bk_sb[:, m : m + 1]
        )

    def v_unit(t):
        tsl = slice(128 * (t % 4), 128 * (t % 4 + 1))
        vp = psmisc.tile([128, SBW], F32, tag="misc")
        for e in range(NE):
            nc.tensor.matmul(
                vp[:, 0:HD], hs_sb[t // 4][:, e, tsl], wv_sb[:, e, :],
                start=(e == 0), stop=(e == NE - 1),
            )
        nc.vector.tensor_add(
            out=v_sb[t][:],
            in0=vp[:, 0:HD].rearrange("p (h d) -> p h d", h=NH),
            in1=bv_bc[:].rearrange("p (h d) -> p h d", h=NH),
        )

    pb_tiles = [None] * NE

    def part_unit(et):
        """f=0,1 partial of the final s-block's out-proj, bias folded in."""
        ssl = slice(SBW * (NS - 1), SBW * NS)
        esl = slice(128 * et, 128 * (et + 1))
        op = psmisc.tile([128, SBW], F32, tag="misc")
        for f in range(2):
            nc.tensor.matmul(
                op[:], woT_sb[:, f, esl], ctxT_sb[f][:, ssl],
                start=(f == 0), stop=(f == 1),
            )
        pb = pbp.tile([128, SBW], F32, tag="pb")
        nc.vector.tensor_scalar_add(
            out=pb[:], in0=op[:], scalar1=bo_sb[:, et : et + 1]
        )
        pb_tiles[et] = pb

    def fin_unit(et):
        """f=2 of the final s-block + add to the partial, then DMA."""
        ssl = slice(SBW * (NS - 1), SBW * NS)
        esl = slice(128 * et, 128 * (et + 1))
        op = psmisc.tile([128, SBW], F32, tag="misc")
        nc.tensor.matmul(
            op[:], woT_sb[:, 2, esl], ctxT_sb[2][:, ssl],
            start=True, stop=True,
        )
        ob = outp.tile([128, SBW], F32, tag="ob")
        nc.vector.tensor_add(out=ob[:], in0=pb_tiles[et][:], in1=op[:])
        nc.sync.dma_start(outT[esl, ssl], ob[:])

    def out_unit(s, et):
        esl = slice(128 * et, 128 * (et + 1))
        ssl = slice(SBW * s, SBW * (s + 1))
        op = psmisc.tile([128, SBW], F32, tag="misc")
        for f in range(NM):
            nc.tensor.matmul(
                op[:], woT_sb[:, f, esl], ctxT_sb[f][:, ssl],
                start=(f == 0), stop=(f == NM - 1),
            )
        ob = outp.tile([128, SBW], F32, tag="ob")
        nc.vector.tensor_scalar_add(
            out=ob[:], in0=op[:], scalar1=bo_sb[:, et : et + 1]
        )
        nc.sync.dma_start(outT[esl, ssl], ob[:])

    # --- lead-in: just the two chains the first score tiles need ---
    k_unit(0, 0)
    q_unit(0, 0)

    bg = deque()          # v tiles just-in-time: v[t] issued at step t
    for t in range(NT):
        bg.append(lambda t=t: v_unit(t))
    # bg2 in deadline order (combo 0 consumes kT[0] fully at steps 4/8/12;
    # combo c needs qT[0][:, c*SBW:] at its step 0)
    bg2 = deque()
    bg2.append(lambda: k_unit(0, 1))
    bg2.append(lambda: k_unit(0, 2))
    bg2.append(lambda: q_unit(0, 1))
    bg2.append(lambda: k_unit(0, 3))
    bg2.append(lambda: q_unit(0, 2))
    bg2.append(lambda: q_unit(0, 3))
    for m in (1, 2):
        for s in range(NS):
            bg2.append(lambda m=m, s=s: k_unit(m, s))
        for s in range(NS):
            bg2.append(lambda m=m, s=s: q_unit(m, s))
    bgout = deque()       # fused out-proj units
    bgpart = deque()      # final-s-block partial out-proj units

    # --- phase 2 ---
    pending_norm = [None]  # deferred normalize-scale of the previous pair

    def norm_scale(p, ssl, cu):
        """zs quadrants hold Z (replicated x32). Selector matmuls assemble the
        per-partition broadcast into psum, one base-0 reciprocal inverts it,
        one [128,512] multiply scales the evacuated ctx into ctxT."""
        bcz = psmisc.tile([128, SBW], F32, tag="misc")
        nc.tensor.matmul(bcz[0:64, 0:HB], sels[0][:, :], zs[:, 0:HB],
                         start=True, stop=True, skip_group_check=True)
        nc.tensor.matmul(bcz[64:128, 0:HB], sels[2][:, :], zs[:, 0:HB],
                         start=True, stop=True, skip_group_check=True)
        nc.tensor.matmul(bcz[0:64, HB:SBW], sels[1][:, :], zs[:, HB:SBW],
                         start=True, stop=True, skip_group_check=True)
        nc.tensor.matmul(bcz[64:128, HB:SBW], sels[3][:, :], zs[:, HB:SBW],
                         start=True, stop=True, skip_group_check=True)
        bci = bcip.tile([128, SBW], F32, tag="bci")
        c = RECIP_APPROX_FAST_CONSTS
        nc.vector._custom_dve(RECIPROCAL_APPROX_FAST, out=bci[:], in0=bcz[:],
                              s0=c["s0"], s1=c["s1"], imm2=c["imm2"])
        nc.vector.tensor_mul(out=ctxT_sb[p][:, ssl], in0=cu[:], in1=bci[:])

    carry = [None]  # finishes the previous pair: flush trailing ctx/den,
    # reciprocals off den psum, evacuate ctx psum, queue norm

    # p-outer sweep: head-pair deadlines for background qk chains land 4x
    # later than s-outer, so the chain work spreads across the whole span
    for pi, (s, p) in enumerate([(s, p) for p in range(NM) for s in range(NS)]):
        ssl = slice(SBW * s, SBW * (s + 1))
        kTh = kT_sb[p]
        qTh = qT_sb[p]
        cps = []   # [ctx_bank, den_bank], allocated lazily at first use
        pend = deque()  # depth-PEND: ctx/den consume exp from PEND steps ago

        es_l1 = [None, None]
        es_l2 = [None]

        def ctx_mms(ex, t, stop, p=p, cps=cps, pend=pend, es_l1=es_l1, es_l2=es_l2):
            if not cps:
                cps.append(psctx.tile([128, SBW], F32, tag="ctx", name="cp"))
                cps.append(psden.tile([128, SBW], F32, tag="den", name="dn"))
            cp, dn = cps
            st = t == 0
            # two heads col-tiled into one bank (tile_position auto-derived
            # from out base partition: (0,0) and (0,64))
            nc.tensor.matmul(
                cp[0:64, :], v_sb[t][:, 2 * p, :], ex[:, 0:SBW],
                start=st, stop=stop, skip_group_check=True,
            )
            nc.tensor.matmul(
                cp[64:128, :], v_sb[t][:, 2 * p + 1, :], ex[:, SBW : 2 * SBW],
                start=st, stop=stop, skip_group_check=True,
            )
            # denominators: DVE sums pairs then quads of exp tiles; one
            # 4-way col-tiled M=32 ones-matmul group per FOUR steps. The
            # den group for quad q runs at the first pop of quad q+1 (one
            # step of slack on the level-2 add), the last quad in finish.
            def den_group(es, std, stp):
                nc.tensor.matmul(
                    dn[0:32, 0:HB], ones32[:, :], es[:, 0:HB],
                    start=std, stop=stp, skip_group_check=True,
                )
                nc.tensor.matmul(
                    dn[32:64, HB:SBW], ones32[:, :], es[:, HB:SBW],
                    start=std, stop=stp, skip_group_check=True,
                )
                nc.tensor.matmul(
                    dn[64:96, 0:HB], ones32[:, :], es[:, SBW : SBW + HB],
                    start=std, stop=stp, skip_group_check=True,
                )
                nc.tensor.matmul(
                    dn[96:128, HB:SBW], ones32[:, :], es[:, SBW + HB : 2 * SBW],
                    start=std, stop=stp, skip_group_check=True,
                    tile_position=(0, 96),
                )

            if t % 4 == 0 and t > 0:
                den_group(es_l2[0], t == 4, False)
            if t % 2 == 0:
                nxt = pend[0][0]  # ex(t+1), already issued
                es = esump.tile([128, 2 * SBW], BF16, tag="es")
                nc.vector.tensor_add(out=es[:], in0=ex[:], in1=nxt[:])
                es_l1[(t // 2) % 2] = es
            elif t % 4 == 3:
                es = esump.tile([128, 2 * SBW], BF16, tag="es2")
                nc.vector.tensor_add(out=es[:], in0=es_l1[0][:], in1=es_l1[1][:])
                es_l2[0] = es
                if t == NT - 1:
                    den_group(es, False, True)

        for t in range(NT):
            tsl = slice(128 * t, 128 * (t + 1))
            sc = pssc.tile([128, 2 * SBW], F32, tag="sc")
            nc.tensor.matmul(
                sc[:, 0:SBW], kTh[0:D, tsl], qTh[0:D, ssl],
                start=True, stop=True,
            )
            nc.tensor.matmul(
                sc[:, SBW : 2 * SBW], kTh[D:128, tsl], qTh[D:128, ssl],
                start=True, stop=True,
            )
            if t == 0 and carry[0] is not None:
                carry[0]()
                carry[0] = None
            if len(pend) == PEND:
                ex2, t2 = pend.popleft()
                ctx_mms(ex2, t2, stop=False)
            ex = expp.tile([128, 2 * SBW], BF16, tag="exp")
            nc.scalar.activation(ex[:], sc[:], AF.Exp)
            pend.append((ex, t))
            if t == 3 and pending_norm[0] is not None:
                pending_norm[0]()
                pending_norm[0] = None
            # background fill: v just-in-time; bg2 chains every other step;
            # out-proj in the remaining slots
            if bg:
                bg.popleft()()
            if bg2 and t % 2 == 0 and t < 14:
                bg2.popleft()()
            elif bgpart and 6 <= t < 14:
                bgpart.popleft()()
            elif bgout and 6 <= t < 14:  # after this pair's t==3 norm_scale
                bgout.popleft()()
            # last combo: drain the ctx/den lag early so the final norm
            # starts right after the last exp
            if pi == NS * NM - 1 and t >= 13 and pend:
                ex2, t2 = pend.popleft()
                ctx_mms(ex2, t2, stop=False)

        def finish_pair(pend=pend, ctx_mms=ctx_mms, cps=cps, p=p, ssl=ssl,
                        last=(pi == NS * NM - 1)):
            while pend:
                ex2, t2 = pend.popleft()
                ctx_mms(ex2, t2, stop=(t2 == NT - 1))
            cp, dn = cps
            # stock same-base psum->sbuf copies of the Z quadrants (bf16);
            # in the drain the scalar engine is idle, so split the casts
            if last:
                nc.scalar.copy(zs[0:32, 0:HB], dn[0:32, 0:HB])
                nc.scalar.copy(zs[32:64, HB:SBW], dn[32:64, HB:SBW])
            else:
                nc.vector.tensor_copy(zs[0:32, 0:HB], dn[0:32, 0:HB])
                nc.vector.tensor_copy(zs[32:64, HB:SBW], dn[32:64, HB:SBW])
            nc.vector.tensor_copy(zs[64:96, 0:HB], dn[64:96, 0:HB])
            nc.vector.tensor_copy(zs[96:128, HB:SBW], dn[96:128, HB:SBW])
            cu = cup.tile([128, SBW], F32, tag="cu")
            nc.vector.tensor_copy(cu[:], cp[:])
            pending_norm[0] = lambda: norm_scale(p, ssl, cu)

        carry[0] = finish_pair
        if pi == 2 * NS - 1:  # pair (3,1) queued: s=3 f0/f1 partials become
            for et in range(NE):  # runnable once its norm lands next combo
                bgpart.append(lambda et=et: part_unit(et))
        if p == NM - 1 and s != NS - 1:
            for et in range(NE):
                bgout.append(lambda s=s, et=et: out_unit(s, et))

    # --- drain: final pair's flush + normalize, then the six f=2 finishes ---
    carry[0]()
    pending_norm[0]()
    while bg2:
        bg2.popleft()()
    while bgpart:
        bgpart.popleft()()
    while bgout:
        bgout.popleft()()
    for et in range(NE):
        fin_unit(et)


def build_nc():
    nc = bacc.Bacc("TRN2", target_bir_lowering=False, debug=False)
    hs = nc.dram_tensor("hs", [128, NS, NE, SBW], BF16, kind="ExternalInput")
    wq = nc.dram_tensor("wq", [128, NE, HD], BF16, kind="ExternalInput")
    wk = nc.dram_tensor("wk", [128, NE, HD], BF16, kind="ExternalInput")
    wv = nc.dram_tensor("wv", [128, NE, HD], BF16, kind="ExternalInput")
    bq = nc.dram_tensor("bq", [HD], F32, kind="ExternalInput")
    bk = nc.dram_tensor("bk", [HD], F32, kind="ExternalInput")
    bv = nc.dram_tensor("bv", [HD], F32, kind="ExternalInput")
    woT = nc.dram_tensor("woT", [128, NM, E], BF16, kind="ExternalInput")
    bo2 = nc.dram_tensor("bo2", [E], F32, kind="ExternalInput")
    outT = nc.dram_tensor("outT", [E, S], F32, kind="ExternalOutput")

    with tile.TileContext(nc) as tc:
        mha_tile(
            tc,
            hs[:, :, :, :], wq[:, :, :], wk[:, :, :], wv[:, :, :],
            bq[:], bk[:], bv[:],
            woT[:, :, :], bo2[:], outT[:, :],
        )
    nc.compile()
    return nc


def _pack(x: np.ndarray, nchunk: int, dtype=ml_dtypes.bfloat16) -> np.ndarray:
    """[nchunk*128, cols] row-major -> [128, nchunk, cols] (partition-major)."""
    cols = x.shape[1]
    return np.ascontiguousarray(
        x.reshape(nchunk, 128, cols).transpose(1, 0, 2).astype(dtype)
    )


def make_core_inputs(inputs: dict) -> list[dict]:
    """Full inputs -> per-core input maps (core c: batch c//2, head-group c%2)."""
    hsf = np.ascontiguousarray(np.asarray(inputs["hidden_state"], dtype=np.float32))
    Wq = np.asarray(inputs["Wq"], dtype=np.float32)
    Wk = np.asarray(inputs["Wk"], dtype=np.float32)
    Wv = np.asarray(inputs["Wv"], dtype=np.float32)
    Wo = np.asarray(inputs["Wo"], dtype=np.float32)
    bq = np.asarray(inputs["bq"], dtype=np.float32)
    bk = np.asarray(inputs["bk"], dtype=np.float32)
    bv = np.asarray(inputs["bv"], dtype=np.float32)
    bo = np.asarray(inputs["bo"], dtype=np.float32)

    maps = []
    for c in range(8):
        b, g = c // 2, c % 2
        hsl = slice(NH * g, NH * (g + 1))
        fsl = slice(HD * g, HD * (g + 1))
        maps.append(
            {
                "hs": np.ascontiguousarray(
                    hsf[b].reshape(NE, 128, NS, SBW)
                    .transpose(1, 2, 0, 3)
                    .astype(ml_dtypes.bfloat16)
                ),
                "wq": _pack(Wq[hsl].transpose(1, 0, 2).reshape(E, HD), NE),
                "wk": _pack(Wk[hsl].transpose(1, 0, 2).reshape(E, HD), NE),
                "wv": _pack(Wv[hsl].transpose(1, 0, 2).reshape(E, HD), NE),
                "bq": np.ascontiguousarray(bq[hsl].reshape(HD)),
                "bk": np.ascontiguousarray(bk[hsl].reshape(HD)),
                "bv": np.ascontiguousarray(bv[hsl].reshape(HD)),
                "woT": _pack(np.ascontiguousarray(Wo[:, fsl].T), NM),
                "bo2": np.ascontiguousarray(bo / 2.0),
            }
        )
    return maps


def combine_outputs(core_outs: list) -> np.ndarray:
    """Per-core outT partials -> full [B, E, S] output."""
    return np.stack(
        [core_outs[2 * b]["outT"] + core_outs[2 * b + 1]["outT"] for b in range(B)]
    ).astype(np.float32)


from concourse.bass_utils import run_bass_kernel_spmd

N_CORES = 8
_NC_CACHE = None


def _get_nc():
    global _NC_CACHE
    if _NC_CACHE is None:
        _NC_CACHE = build_nc()
    return _NC_CACHE


def kernel(**inputs) -> np.ndarray:
    """Full-input entry point: shard across 8 cores, run, unshard."""
    maps = make_core_inputs(inputs)
    nc = _get_nc()
    res = run_bass_kernel_spmd(nc, maps, core_ids=list(range(N_CORES)))
    outs = res.results
    return np.stack(
        [outs[2 * b]["outT"] + outs[2 * b + 1]["outT"] for b in range(B)]
    ).astype(np.float32)
